# revision 10
# baseline (speedup 1.0000x reference)
"""Trainium2 Bass kernel for nn_LinearMultiheadAttention (linear attention with
polynomial feature map phi(x) = [1, x, 0.5 x^2]), sharded over 8 NeuronCores.

Sharding: core c -> batch b = c//2, heads h0 = (c%2)*8 .. h0+8.
Each core computes a partial output (its 8 heads' contribution through Wo);
the host sums the two partials per batch.

Precision: the z = qsum*ksum normalizer is ill-conditioned (min |qsum| ~3e-4
while outputs reach ~4e4), so the q projection is computed to full fp32
accuracy via an exact 3-term fp32r split (hs = hi + lo, Wq = Whi + Wlo,
products exact in the fp32 PSUM accumulate; only the lo*lo term ~2^-24 is
dropped).  The k projection runs a single bf16 pass: the only k-quantity with
catastrophic cancellation is the *linear* part of ksum = sum_n phi(k_n), and
that equals (sum_n hs_n) @ Wk, which is computed exactly from an fp32
token-sum of hs (DVE reduction of the transposed tiles) and a tiny 3-term
f32r matvec in the mid phase.  The k^2 slots of ksum have no cancellation
(sums of squares), so bf16 k is plenty.  v / kv / qkv / Wo run at bf16.

hs arrives from the host already transposed (hsT [1152, 4096], zero-padded
from 1040 rows) so pass A needs no PE transposes at all.  Pass A is software
pipelined: iteration t emits load+derive(t), projections(t-1),
kv-accumulation(t-2) so the in-order PE stream never waits on Act/DVE phi
assembly.  Pass B similarly lags each PE stage by one tile.
"""
import numpy as np
import ml_dtypes

import concourse.bass as bass
import concourse.tile as tile
from concourse import bacc, mybir
from concourse.bass_utils import run_bass_kernel_spmd

F32 = mybir.dt.float32
F32R = mybir.dt.float32r
BF16 = mybir.dt.bfloat16
AF = mybir.ActivationFunctionType
ALU = mybir.AluOpType

B, S, D = 4, 4096, 1040
H, F, E = 16, 32, 65          # heads, feature_dim, head_dim (= 2F+1)
HPC = 8                        # heads per core
P = 128
NT = S // P                    # 32 token tiles per core
NCH = 9                        # ceil(D/128); last chunk K=16
DP = NCH * P                   # 1152 = zero-padded D
KLAST = D - 8 * P              # 16
QW = HPC * F                   # 256 q cols per core
KW = HPC * F                   # 256 k cols per core
VW = HPC * E                   # 520 v cols per core
KVW = KW + VW                  # 776
OCH = 5                        # ceil(VW/128); last chunk K=8
OLAST = VW - 4 * P             # 8
RSQRT2 = float(1.0 / np.sqrt(2.0))

_CACHED = {}


def _chunk_k(c):
    return KLAST if c == NCH - 1 else P


def build_bass():
    nc = bacc.Bacc("TRN2", target_bir_lowering=False, debug=False, num_devices=8)
    hsT = nc.dram_tensor("hsT", [NCH, P, S], F32, kind="ExternalInput").ap()
    maskf = nc.dram_tensor("maskf", [P, NT], F32, kind="ExternalInput").ap()
    wq_hilo = nc.dram_tensor("wq_hilo", [NCH, P, 2 * QW], F32R,
                             kind="ExternalInput").ap()
    wk_hilo = nc.dram_tensor("wk_hilo", [NCH, P, 2 * KW], F32R,
                             kind="ExternalInput").ap()
    wkv = nc.dram_tensor("wkv", [NCH, P, KVW], BF16, kind="ExternalInput").ap()
    wo = nc.dram_tensor("wo", [OCH, P, D], BF16, kind="ExternalInput").ap()
    id32 = nc.dram_tensor("id32", [P, P], F32, kind="ExternalInput").ap()
    id16 = nc.dram_tensor("id16", [P, P], BF16, kind="ExternalInput").ap()
    ones_col = nc.dram_tensor("ones_col", [P, 1], F32, kind="ExternalInput").ap()
    out = nc.dram_tensor("out", [S, D], F32, kind="ExternalOutput").ap()

    with tile.TileContext(nc) as tc:
        with (
            tc.tile_pool(name="consts", bufs=1) as consts,
            tc.tile_pool(name="state", bufs=1) as state,
            tc.tile_pool(name="rot2", bufs=2) as rot2,
            tc.tile_pool(name="rot3", bufs=3) as rot3,
            tc.tile_pool(name="ps", bufs=1, space="PSUM") as ps,
            tc.tile_pool(name="ps2", bufs=2, space="PSUM") as ps2,
        ):
            # ---- constants needed for pass A (DMA'd after hsT tile 0 so the
            # first tile's derivation overlaps the weight transfers) ----
            mask_sb = consts.tile([P, NT], F32)
            wqh_sb = consts.tile([P, NCH, 2 * QW], F32R)
            wkv_sb = consts.tile([P, NCH, KVW], BF16)

            def emit_early_consts():
                nc.sync.dma_start(out=mask_sb, in_=maskf)
                nc.sync.dma_start(out=wqh_sb,
                                  in_=wq_hilo.rearrange("c p j -> p c j"))
                nc.sync.dma_start(out=wkv_sb,
                                  in_=wkv.rearrange("c p j -> p c j"))

            # ---- persistent state ----
            phiq = state.tile([P, NT, HPC, E], BF16)   # rq-folded phi(q) stash
            hssum_acc = state.tile([P, NCH], F32)      # sum_n hs (exact fp32)
            sqk_acc = state.tile([P, KW], F32)         # sum_tiles k^2 partials
            kvs_sb = state.tile([E, VW], BF16)         # rk-scaled kv
            ksum_row = state.tile([1, VW], F32)
            rk_row = state.tile([1, VW], F32)
            rk_bc = state.tile([P, VW], F32)
            nc.vector.memset(hssum_acc[:], 0.0)
            nc.vector.memset(sqk_acc[:], 0.0)
            kv_acc = [ps.tile([P, 4 * E], F32, tag=f"a{i}", name=f"kv{i}")
                      for i in range(2)]

            hst_t = {}
            hsr_t = {}
            hlo_t = {}
            h16_t = {}
            phik_t = {}
            v16_t = {}

            def stage_T(t):
                """DMA transposed tile t; derive f32r hi/lo + bf16; hssum."""
                hsT_sb = rot3.tile([P, NCH, P], F32, tag="hs")
                nc.sync.dma_start(
                    out=hsT_sb, in_=hsT[:, :, t * P:(t + 1) * P].rearrange(
                        "c p n -> p c n"))
                hst_t[t] = hsT_sb
                hsr = rot2.tile([P, NCH, P], F32R, tag="hsr")
                hlo = rot2.tile([P, NCH, P], F32R, tag="hlo")
                h16 = rot2.tile([P, NCH, P], BF16, tag="h16")
                flat = hsT_sb[:].rearrange("p c n -> p (c n)")
                nc.scalar.activation(hsr[:].rearrange("p c n -> p (c n)"),
                                     flat, AF.Copy)
                nc.vector.tensor_tensor(hlo[:].rearrange("p c n -> p (c n)"),
                                        flat,
                                        hsr[:].rearrange("p c n -> p (c n)")
                                        .bitcast(F32), ALU.subtract)
                nc.scalar.activation(h16[:].rearrange("p c n -> p (c n)"),
                                     flat, AF.Copy)
                hsr_t[t], hlo_t[t], h16_t[t] = hsr, hlo, h16

            def stage_P(t):
                """Projections + phi assembly for tile t (inputs ready)."""
                hsT_sb = hst_t.pop(t)
                hsr, hlo, h16 = hsr_t.pop(t), hlo_t.pop(t), h16_t.pop(t)
                # hssum: exact fp32 token-sums (off critical path)
                hst = rot2.tile([P, NCH], F32, tag="hst")
                nc.vector.tensor_reduce(hst[:], hsT_sb[:],
                                        mybir.AxisListType.X, ALU.add)
                nc.vector.tensor_add(hssum_acc[:], hssum_acc[:], hst[:])
                # q: 3-term f32r. hi @ [Whi|Wlo] then lo @ Whi into [0:QW].
                q_ps = ps2.tile([P, 2 * QW], F32, tag="qk", name=f"q_{t}")
                for c in range(NCH):
                    kk = _chunk_k(c)
                    nc.tensor.matmul(q_ps[:], hsr[0:kk, c, :], wqh_sb[0:kk, c, :],
                                     start=(c == 0), stop=False,
                                     skip_group_check=True)
                for c in range(NCH):
                    kk = _chunk_k(c)
                    nc.tensor.matmul(q_ps[:, 0:QW], hlo[0:kk, c, :],
                                     wqh_sb[0:kk, c, 0:QW],
                                     start=False, stop=(c == NCH - 1),
                                     skip_group_check=True)
                # k|v single bf16 pass
                kv1_ps = ps2.tile([P, 512], F32, tag="kv1", name=f"kv1_{t}")
                kv2_ps = ps2.tile([P, KVW - 512], F32, tag="kv2",
                                  name=f"kv2_{t}")
                for c in range(NCH):
                    kk = _chunk_k(c)
                    nc.tensor.matmul(kv1_ps[:], h16[0:kk, c, :],
                                     wkv_sb[0:kk, c, 0:512],
                                     start=(c == 0), stop=(c == NCH - 1))
                    nc.tensor.matmul(kv2_ps[:], h16[0:kk, c, :],
                                     wkv_sb[0:kk, c, 512:KVW],
                                     start=(c == 0), stop=(c == NCH - 1))

                # ---- q side (DVE) ----
                qh = rot2.tile([P, QW], F32, tag="qh")
                nc.scalar.activation(qh[:], q_ps[:, QW:2 * QW], AF.Copy)
                qf = rot2.tile([P, QW], F32, tag="qf")
                nc.vector.tensor_tensor(qf[:], q_ps[:, 0:QW], qh[:], ALU.add)
                sq2 = rot2.tile([P, QW], F32, tag="sq2")
                nc.vector.tensor_mul(sq2[:], qf[:], qf[:])
                sumq = rot2.tile([P, HPC], F32, tag="sumq")
                nc.vector.tensor_reduce(
                    sumq[:], qf[:].rearrange("p (h f) -> p h f", f=F),
                    mybir.AxisListType.X, ALU.add)
                sumq2 = rot2.tile([P, HPC], F32, tag="sumq2")
                nc.vector.tensor_reduce(
                    sumq2[:], sq2[:].rearrange("p (h f) -> p h f", f=F),
                    mybir.AxisListType.X, ALU.add)
                qsum = rot2.tile([P, HPC], F32, tag="qsum")
                nc.vector.tensor_scalar(qsum[:], sumq2[:], 0.5, 1.0,
                                        ALU.mult, ALU.add)
                nc.vector.tensor_add(qsum[:], qsum[:], sumq[:])
                rq = rot2.tile([P, HPC], F32, tag="rq")
                nc.vector.reciprocal(rq[:], qsum[:])
                nc.vector.tensor_mul(
                    rq[:], rq[:], mask_sb[:, t:t + 1].broadcast_to([P, HPC]))
                rq05 = rot2.tile([P, HPC], F32, tag="rq05")
                nc.vector.tensor_scalar_mul(rq05[:], rq[:], 0.5)

                # phi_q stash: slot0 + square slots on gpsimd (off critical
                # path), linear slots on DVE.
                pq = phiq[:, t]
                nc.gpsimd.tensor_copy(pq[:, :, 0:1], rq[:].unsqueeze(2))
                nc.vector.tensor_mul(
                    pq[:, :, 1:1 + F],
                    qf[:].rearrange("p (h f) -> p h f", f=F),
                    rq[:].unsqueeze(2).broadcast_to([P, HPC, F]))
                nc.gpsimd.tensor_mul(
                    pq[:, :, 1 + F:E],
                    sq2[:].rearrange("p (h f) -> p h f", f=F),
                    rq05[:].unsqueeze(2).broadcast_to([P, HPC, F]))

                # ---- k side (Act) ----
                phik = rot2.tile([P, HPC, E], BF16, tag="phik")
                nc.vector.memset(phik[:, :, 0:1], 1.0)
                nc.scalar.activation(
                    phik[:, :, 1:1 + F],
                    kv1_ps[:, 0:KW].rearrange("p (h f) -> p h f", f=F), AF.Copy)
                nc.scalar.activation(
                    phik[:, :, 1 + F:E],
                    kv1_ps[:, 0:KW].rearrange("p (h f) -> p h f", f=F),
                    AF.Square, scale=RSQRT2)
                # k^2 accumulation for ksum (fp32, off critical path)
                sqk = rot2.tile([P, KW], F32, tag="sqk")
                nc.scalar.activation(sqk[:], kv1_ps[:, 0:KW], AF.Square)
                nc.gpsimd.tensor_add(sqk_acc[:], sqk_acc[:], sqk[:])
                # v16
                v16 = rot2.tile([P, VW], BF16, tag="v16")
                nc.scalar.activation(v16[:, 0:512 - KW], kv1_ps[:, KW:512],
                                     AF.Copy)
                nc.scalar.activation(v16[:, 512 - KW:VW], kv2_ps[:], AF.Copy)
                phik_t[t], v16_t[t] = phik, v16

            def stage_KV(t):
                """kv accumulation matmuls for tile t."""
                phik, v16 = phik_t.pop(t), v16_t.pop(t)
                for h in range(HPC):
                    nc.tensor.matmul(
                        kv_acc[h // 4][0:E, (h % 4) * E:(h % 4) * E + E],
                        phik[:, h, :], v16[:, h * E:h * E + E],
                        start=(t == 0 and h % 4 == 0), stop=(t == NT - 1),
                        skip_group_check=True)

            # late constants (consumed in mid / pass B) are DMA'd from the
            # middle of pass A so they don't delay the first projections.
            late_consts = {}

            def emit_late_consts():
                wkh_sb = consts.tile([P, NCH, 2 * KW], F32R)
                nc.sync.dma_start(out=wkh_sb,
                                  in_=wk_hilo.rearrange("c p j -> p c j"))
                wo_sb = consts.tile([P, OCH, D], BF16)
                nc.sync.dma_start(out=wo_sb,
                                  in_=wo.rearrange("c p j -> p c j"))
                id32_sb = consts.tile([P, P], F32)
                nc.sync.dma_start(out=id32_sb, in_=id32)
                id16_sb = consts.tile([P, P], BF16)
                nc.sync.dma_start(out=id16_sb, in_=id16)
                ones_sb = consts.tile([P, 1], F32)
                nc.sync.dma_start(out=ones_sb, in_=ones_col)
                late_consts.update(wkh_sb=wkh_sb, wo_sb=wo_sb, id32_sb=id32_sb,
                                   id16_sb=id16_sb, ones_sb=ones_sb)

            # =============== PASS A (software pipelined) ===============
            for t in range(NT + 2):
                with nc.named_scope(f"A{t}"):
                    if t == NT // 2:
                        emit_late_consts()
                    if t < NT:
                        stage_T(t)
                    if t == 0:
                        emit_early_consts()
                    if 1 <= t <= NT:
                        stage_P(t - 1)
                    if t >= 2:
                        stage_KV(t - 2)

            wkh_sb = late_consts["wkh_sb"]
            wo_sb = late_consts["wo_sb"]
            id32_sb = late_consts["id32_sb"]
            id16_sb = late_consts["id16_sb"]
            ones_sb = late_consts["ones_sb"]

            # =============== MID: ksum assembly ===============
            with nc.named_scope("mid"):
                hs_hi = state.tile([P, NCH], F32R)
                hs_lo = state.tile([P, NCH], F32R)
                nc.vector.tensor_copy(hs_hi[:], hssum_acc[:])
                nc.vector.tensor_tensor(hs_lo[:], hssum_acc[:],
                                        hs_hi[:].bitcast(F32), ALU.subtract)
                # ksum linear slots: 3-term f32r matvec hssum @ Wk
                kl_ps = ps2.tile([1, KW], F32, tag="qk", name="kl")
                for pi, (vec, off) in enumerate(
                        [(hs_hi, 0), (hs_hi, KW), (hs_lo, 0)]):
                    for c in range(NCH):
                        kk = _chunk_k(c)
                        nc.tensor.matmul(
                            kl_ps[:], vec[0:kk, c:c + 1],
                            wkh_sb[0:kk, c, off:off + KW],
                            start=(pi == 0 and c == 0),
                            stop=(pi == 2 and c == NCH - 1),
                            skip_group_check=True)
                # ksum square slots: column-sum sqk_acc over tokens, transpose
                kcol_ps = ps2.tile([P, 2], F32, tag="kv2", name="kcol")
                for j in range(2):
                    nc.tensor.matmul(kcol_ps[:, j:j + 1],
                                     sqk_acc[:, j * P:(j + 1) * P], ones_sb[:],
                                     start=(j == 0), stop=(j == 1),
                                     skip_group_check=True)
                kcol_sb = state.tile([P, 2], F32)
                nc.vector.tensor_copy(kcol_sb[:], kcol_ps[:])
                ktr_ps = ps2.tile([2, P], F32, tag="kv1", name="ktr")
                nc.tensor.transpose(ktr_ps[:], kcol_sb[:], id32_sb[:])
                kt = state.tile([2, P], F32)
                nc.vector.tensor_scalar_mul(kt[:], ktr_ps[:], 0.5)

                krow = ksum_row[:].rearrange("o (h e) -> o h e", e=E)
                nc.vector.memset(krow[:, :, 0:1], float(S))
                nc.vector.tensor_copy(
                    krow[:, :, 1:1 + F],
                    kl_ps[:].rearrange("o (h f) -> o h f", f=F))
                nc.vector.tensor_copy(
                    krow[:, 0:4, 1 + F:E],
                    kt[0:1, :].rearrange("o (h f) -> o h f", f=F))
                nc.sync.dma_start(
                    out=krow[:, 4:8, 1 + F:E],
                    in_=kt[1:2, :].rearrange("o (h f) -> o h f", f=F))
                nc.vector.reciprocal(rk_row[:], ksum_row[:])
                nc.gpsimd.partition_broadcast(rk_bc[:], rk_row[:])
                for i in range(2):
                    nc.vector.tensor_mul(
                        kvs_sb[:, i * 4 * E:(i + 1) * 4 * E],
                        kv_acc[i][0:E, :],
                        rk_bc[0:E, i * 4 * E:(i + 1) * 4 * E])

            # =============== PASS B (software pipelined) ===============
            phiT_t = {}
            osb_t = {}
            oT_t = {}

            def stage_BT(t):
                phiT = rot2.tile([E, HPC, P], BF16, tag="phiT")
                for g in range(2):
                    tp = ps2.tile([P, 512], BF16, tag="kv1", name=f"ptp_{t}_{g}")
                    for hh in range(4):
                        h = g * 4 + hh
                        nc.tensor.transpose(tp[0:E, hh * P:hh * P + P],
                                            phiq[:, t, h, :], id16_sb[:])
                    nc.vector.tensor_copy(
                        phiT[:, g * 4:(g + 1) * 4, :].rearrange(
                            "p h n -> p (h n)"),
                        tp[0:E, :])
                phiT_t[t] = phiT

            def stage_BQ(t):
                phiT = phiT_t.pop(t)
                o_ps = [ps.tile([P, 4 * E], F32, tag="a0", name=f"o0_{t}"),
                        ps.tile([P, 4 * E], F32, tag="a1", name=f"o1_{t}")]
                for h in range(HPC):
                    nc.tensor.matmul(
                        o_ps[h // 4][:, (h % 4) * E:(h % 4) * E + E],
                        phiT[:, h, :], kvs_sb[:, h * E:h * E + E],
                        start=(h % 4 == 0), stop=(h % 4 == 3),
                        skip_group_check=True)
                o_sb = rot2.tile([P, VW], BF16, tag="osb")
                nc.vector.tensor_copy(o_sb[:, 0:4 * E], o_ps[0][:])
                nc.scalar.activation(o_sb[:, 4 * E:VW], o_ps[1][:], AF.Copy)
                osb_t[t] = o_sb

            def stage_BO(t):
                o_sb = osb_t.pop(t)
                oT = rot2.tile([P, OCH, P], BF16, tag="oT")
                # chunks 0..3: XBAR DMA transpose (2 on sync queue, 2 on the
                # activation hwdge queue) -- frees the PE and the DVE copies
                for c in range(4):
                    eng = nc.sync if c % 2 == 0 else nc.scalar
                    eng.dma_start_transpose(out=oT[:, c, :],
                                            in_=o_sb[:, c * P:(c + 1) * P])
                # 8-col tail stays on the PE
                tp = ps2.tile([P, P], BF16, tag="kv2", name=f"otp_{t}")
                nc.tensor.transpose(tp[0:OLAST, 0:P],
                                    o_sb[:, 4 * P:4 * P + OLAST], id16_sb[:])
                nc.vector.tensor_copy(oT[0:OLAST, 4, :], tp[0:OLAST, 0:P])
                oT_t[t] = oT

            def stage_BW(t):
                oT = oT_t.pop(t)
                f1 = ps2.tile([P, 512], F32, tag="qk", name=f"f1_{t}")
                f2 = ps2.tile([P, 512], F32, tag="qk", name=f"f2_{t}")
                f3 = ps2.tile([P, D - 1024], F32, tag="kv2", name=f"f3_{t}")
                for c in range(OCH):
                    kk = OLAST if c == OCH - 1 else P
                    nc.tensor.matmul(f1[:], oT[0:kk, c, :],
                                     wo_sb[0:kk, c, 0:512],
                                     start=(c == 0), stop=(c == OCH - 1))
                    nc.tensor.matmul(f2[:], oT[0:kk, c, :],
                                     wo_sb[0:kk, c, 512:1024],
                                     start=(c == 0), stop=(c == OCH - 1))
                    nc.tensor.matmul(f3[:], oT[0:kk, c, :],
                                     wo_sb[0:kk, c, 1024:D],
                                     start=(c == 0), stop=(c == OCH - 1))
                out_sb = rot2.tile([P, D], F32, tag="outsb")
                nc.vector.tensor_copy(out_sb[:, 0:512], f1[:])
                nc.scalar.activation(out_sb[:, 512:1024], f2[:], AF.Copy)
                nc.vector.tensor_copy(out_sb[:, 1024:D], f3[:])
                nc.sync.dma_start(out=out[t * P:(t + 1) * P, :], in_=out_sb)

            for t in range(NT + 3):
                with nc.named_scope(f"B{t}"):
                    if t < NT:
                        stage_BT(t)
                    if 1 <= t <= NT:
                        stage_BQ(t - 1)
                    if 2 <= t <= NT + 1:
                        stage_BO(t - 2)
                    if t >= 3:
                        stage_BW(t - 3)

    nc.compile()
    return nc


def _r12(x):
    """Round fp32 mantissa to 12 explicit bits (safe under PE f32r reads)."""
    xi = np.ascontiguousarray(x, dtype=np.float32).view(np.uint32)
    out = ((xi + np.uint32(0x800)) & np.uint32(0xFFFFF000)).view(np.float32)
    return out.copy()


def _chunks(w):
    out = np.zeros((NCH, P, w.shape[1]), dtype=np.float32)
    for c in range(NCH):
        kk = _chunk_k(c)
        out[c, 0:kk] = w[c * P:c * P + kk]
    return out


def _prep_core_inputs(hidden_states, attention_mask, Wq, Wk, Wv, Wo, core):
    b, half = core // 2, core % 2
    h0 = half * HPC
    bf = ml_dtypes.bfloat16

    # transposed, zero-padded to 1152 rows, chunked [NCH, P, S]
    hsT = np.zeros((NCH, P, S), dtype=np.float32)
    hsTf = np.ascontiguousarray(hidden_states[b].astype(np.float32).T)
    for c in range(NCH):
        kk = _chunk_k(c)
        hsT[c, 0:kk] = hsTf[c * P:c * P + kk]
    maskf = np.ascontiguousarray(
        attention_mask[b].astype(np.float32).reshape(NT, P).T)

    wq_h = Wq[:, h0 * F:(h0 + HPC) * F].astype(np.float32)
    wk_h = Wk[:, h0 * F:(h0 + HPC) * F].astype(np.float32)
    wv_h = Wv[:, h0 * E:(h0 + HPC) * E].astype(np.float32)
    wq_hi = _r12(wq_h)
    wk_hi = _r12(wk_h)
    wq_hilo = _chunks(np.concatenate([wq_hi, wq_h - wq_hi], axis=1))
    wk_hilo = _chunks(np.concatenate([wk_hi, wk_h - wk_hi], axis=1))
    wkv = _chunks(np.concatenate([wk_h, wv_h], axis=1)).astype(bf)

    wo_rows = Wo[h0 * E:(h0 + HPC) * E].astype(np.float32)
    wo_h = np.zeros((OCH, P, D), dtype=np.float32)
    for c in range(OCH):
        kk = OLAST if c == OCH - 1 else P
        wo_h[c, 0:kk] = wo_rows[c * P:c * P + kk]
    wo_h = wo_h.astype(bf)

    return {
        "hsT": hsT,
        "maskf": maskf,
        "wq_hilo": wq_hilo,
        "wk_hilo": wk_hilo,
        "wkv": wkv,
        "wo": wo_h,
        "id32": np.eye(P, dtype=np.float32),
        "id16": np.eye(P, dtype=np.float32).astype(bf),
        "ones_col": np.ones((P, 1), dtype=np.float32),
    }


def kernel(hidden_states, attention_mask, Wq, Wk, Wv, Wo, _trace=False):
    hidden_states = np.asarray(hidden_states)
    attention_mask = np.asarray(attention_mask)
    Wq = np.asarray(Wq); Wk = np.asarray(Wk)
    Wv = np.asarray(Wv); Wo = np.asarray(Wo)

    if "nc" not in _CACHED:
        _CACHED["nc"] = build_bass()
    nc = _CACHED["nc"]

    in_maps = [
        _prep_core_inputs(hidden_states, attention_mask, Wq, Wk, Wv, Wo, c)
        for c in range(8)
    ]
    res = run_bass_kernel_spmd(nc, in_maps, core_ids=list(range(8)),
                               trace=_trace)
    _CACHED["last_result"] = res
    out = np.empty((B, S, D), dtype=np.float32)
    for b in range(B):
        out[b] = res.results[2 * b]["out"] + res.results[2 * b + 1]["out"]
    return out


# revision 15
# speedup vs baseline: 1.3442x; 1.3442x over previous
"""Trainium2 Bass kernel for nn_LinearMultiheadAttention (linear attention with
polynomial feature map phi(x) = [1, x, 0.5 x^2]), sharded over 8 NeuronCores.

Sharding: core c -> batch b = c//2, heads h0 = (c%2)*8 .. h0+8.
Each core computes a partial output (its 8 heads' contribution through Wo);
the host sums the two partials per batch.

Precision: the z = qsum*ksum normalizer is ill-conditioned (min |qsum| ~3e-4
while outputs reach ~4e4), so the q projection is computed to full fp32
accuracy via an exact 3-term fp32r split (hs = hi + lo, Wq = Whi + Wlo,
products exact in the fp32 PSUM accumulate; only the lo*lo term ~2^-24 is
dropped).  The k projection runs a single bf16 pass: the only k-quantity with
catastrophic cancellation is the *linear* part of ksum = sum_n phi(k_n), and
that equals (sum_n hs_n) @ Wk, which is computed exactly from an fp32
token-sum of hs (DVE reduction of the transposed tiles) and a tiny 3-term
f32r matvec in the mid phase.  The k^2 slots of ksum have no cancellation
(sums of squares), so bf16 k is plenty.  v / kv / qkv / Wo run at bf16.

hs arrives from the host already transposed (hsT [1152, 4096], zero-padded
from 1040 rows) so pass A needs no PE transposes at all.  Pass A is software
pipelined: iteration t emits load+derive(t), projections(t-1),
kv-accumulation(t-2) so the in-order PE stream never waits on Act/DVE phi
assembly.  Pass B similarly lags each PE stage by one tile.
"""
import numpy as np
import ml_dtypes

import concourse.bass as bass
import concourse.tile as tile
from concourse import bacc, mybir
from concourse.bass_utils import run_bass_kernel_spmd

F32 = mybir.dt.float32
F32R = mybir.dt.float32r
BF16 = mybir.dt.bfloat16
AF = mybir.ActivationFunctionType
ALU = mybir.AluOpType

B, S, D = 4, 4096, 1040
H, F, E = 16, 32, 65          # heads, feature_dim, head_dim (= 2F+1)
HPC = 8                        # heads per core
P = 128
NT = S // P                    # 32 token tiles per core
NCH = 9                        # ceil(D/128); last chunk K=16
DP = NCH * P                   # 1152 = zero-padded D
KLAST = D - 8 * P              # 16
QW = HPC * F                   # 256 q cols per core
KW = HPC * F                   # 256 k cols per core
VW = HPC * E                   # 520 v cols per core
KVW = KW + VW                  # 776
OCH = 5                        # ceil(VW/128); last chunk K=8
OLAST = VW - 4 * P             # 8
RSQRT2 = float(1.0 / np.sqrt(2.0))

_CACHED = {}


def _chunk_k(c):
    return KLAST if c == NCH - 1 else P


def build_bass():
    nc = bacc.Bacc("TRN2", target_bir_lowering=False, debug=False, num_devices=8)
    hsT = nc.dram_tensor("hsT", [NCH, P, S], F32, kind="ExternalInput").ap()
    maskf = nc.dram_tensor("maskf", [P, NT], F32, kind="ExternalInput").ap()
    wq_hilo = nc.dram_tensor("wq_hilo", [NCH, P, 2 * QW], F32R,
                             kind="ExternalInput").ap()
    wk_hilo = nc.dram_tensor("wk_hilo", [NCH, P, 2 * KW], F32R,
                             kind="ExternalInput").ap()
    wkv = nc.dram_tensor("wkv", [NCH, P, KVW], BF16, kind="ExternalInput").ap()
    wo = nc.dram_tensor("wo", [OCH, P, D], BF16, kind="ExternalInput").ap()
    id32 = nc.dram_tensor("id32", [P, P], F32, kind="ExternalInput").ap()
    id16 = nc.dram_tensor("id16", [P, P], BF16, kind="ExternalInput").ap()
    ones_col = nc.dram_tensor("ones_col", [P, 1], F32, kind="ExternalInput").ap()
    out = nc.dram_tensor("out", [S, D], F32, kind="ExternalOutput").ap()

    with tile.TileContext(nc) as tc:
        with (
            tc.tile_pool(name="consts", bufs=1) as consts,
            tc.tile_pool(name="state", bufs=1) as state,
            tc.tile_pool(name="rot2", bufs=2) as rot2,
            tc.tile_pool(name="rot3", bufs=3) as rot3,
            tc.tile_pool(name="rot4", bufs=4) as rot4,
            tc.tile_pool(name="ps", bufs=1, space="PSUM") as ps,
            tc.tile_pool(name="ps2", bufs=2, space="PSUM") as ps2,
        ):
            # ---- constants needed for pass A (DMA'd after hsT tile 0 so the
            # first tile's derivation overlaps the weight transfers) ----
            mask_sb = consts.tile([P, NT], F32)
            wqh_sb = consts.tile([P, NCH, 2 * QW], F32R)
            wkv_sb = consts.tile([P, NCH, KVW], BF16)

            def emit_early_consts():
                nc.sync.dma_start(out=mask_sb, in_=maskf)
                nc.sync.dma_start(out=wqh_sb,
                                  in_=wq_hilo.rearrange("c p j -> p c j"))
                nc.sync.dma_start(out=wkv_sb,
                                  in_=wkv.rearrange("c p j -> p c j"))

            # ---- persistent state ----
            phiq = state.tile([P, NT, HPC, E], BF16)   # rq-folded phi(q) stash
            hssum_acc = state.tile([P, NCH], F32)      # sum_n hs (exact fp32)
            sqk_acc = state.tile([P, KW], F32)         # sum_tiles k^2 partials
            kvs_sb = state.tile([E, VW], BF16)         # rk-scaled kv
            ksum_row = state.tile([1, VW], F32)
            rk_row = state.tile([1, VW], F32)
            rk_bc = state.tile([P, VW], F32)
            nc.vector.memset(hssum_acc[:], 0.0)
            nc.vector.memset(sqk_acc[:], 0.0)
            kv_acc = [ps.tile([P, 4 * E], F32, tag=f"a{i}", name=f"kv{i}")
                      for i in range(2)]

            hst_t = {}
            hsr_t = {}
            hlo_t = {}
            h16_t = {}
            phik_t = {}
            v16_t = {}

            def stage_T(t):
                """DMA transposed tile t; derive f32r hi/lo + bf16; hssum."""
                hsT_sb = rot3.tile([P, NCH, P], F32, tag="hs")
                nc.sync.dma_start(
                    out=hsT_sb, in_=hsT[:, :, t * P:(t + 1) * P].rearrange(
                        "c p n -> p c n"))
                hst_t[t] = hsT_sb
                hsr = rot2.tile([P, NCH, P], F32R, tag="hsr")
                hlo = rot2.tile([P, NCH, P], F32R, tag="hlo")
                h16 = rot2.tile([P, NCH, P], BF16, tag="h16")
                flat = hsT_sb[:].rearrange("p c n -> p (c n)")
                nc.scalar.activation(hsr[:].rearrange("p c n -> p (c n)"),
                                     flat, AF.Copy)
                nc.vector.tensor_tensor(hlo[:].rearrange("p c n -> p (c n)"),
                                        flat,
                                        hsr[:].rearrange("p c n -> p (c n)")
                                        .bitcast(F32), ALU.subtract)
                nc.scalar.activation(h16[:].rearrange("p c n -> p (c n)"),
                                     flat, AF.Copy)
                hsr_t[t], hlo_t[t], h16_t[t] = hsr, hlo, h16

            def stage_P(t):
                """Projections + phi assembly for tile t (inputs ready)."""
                hsT_sb = hst_t.pop(t)
                hsr, hlo, h16 = hsr_t.pop(t), hlo_t.pop(t), h16_t.pop(t)
                # hssum: exact fp32 token-sums (off critical path)
                hst = rot2.tile([P, NCH], F32, tag="hst")
                nc.vector.tensor_reduce(hst[:], hsT_sb[:],
                                        mybir.AxisListType.X, ALU.add)
                nc.vector.tensor_add(hssum_acc[:], hssum_acc[:], hst[:])
                # q: 3-term f32r. hi @ [Whi|Wlo] then lo @ Whi into [0:QW].
                q_ps = ps2.tile([P, 2 * QW], F32, tag="qk", name=f"q_{t}")
                for c in range(NCH):
                    kk = _chunk_k(c)
                    nc.tensor.matmul(q_ps[:], hsr[0:kk, c, :], wqh_sb[0:kk, c, :],
                                     start=(c == 0), stop=False,
                                     skip_group_check=True)
                # k|v single bf16 pass, interleaved with the q lo-pass so
                # every LDWEIGHTS hides under a long-stream matmul
                kv1_ps = ps2.tile([P, 512], F32, tag="kv1", name=f"kv1_{t}")
                kv2_ps = ps2.tile([P, KVW - 512], F32, tag="kv2",
                                  name=f"kv2_{t}")
                for c in range(NCH):
                    kk = _chunk_k(c)
                    nc.tensor.matmul(kv1_ps[:], h16[0:kk, c, :],
                                     wkv_sb[0:kk, c, 0:512],
                                     start=(c == 0), stop=(c == NCH - 1))
                    nc.tensor.matmul(q_ps[:, 0:QW], hlo[0:kk, c, :],
                                     wqh_sb[0:kk, c, 0:QW],
                                     start=False, stop=(c == NCH - 1),
                                     skip_group_check=True)
                    nc.tensor.matmul(kv2_ps[:], h16[0:kk, c, :],
                                     wkv_sb[0:kk, c, 512:KVW],
                                     start=(c == 0), stop=(c == NCH - 1))

                # ---- q side (DVE) ----
                qh = rot2.tile([P, QW], F32, tag="qh")
                nc.scalar.activation(qh[:], q_ps[:, QW:2 * QW], AF.Copy)
                qf = rot2.tile([P, QW], F32, tag="qf")
                nc.vector.tensor_tensor(qf[:], q_ps[:, 0:QW], qh[:], ALU.add)
                sq2 = rot2.tile([P, QW], F32, tag="sq2")
                nc.vector.tensor_mul(sq2[:], qf[:], qf[:])
                sumq = rot2.tile([P, HPC], F32, tag="sumq")
                nc.vector.tensor_reduce(
                    sumq[:], qf[:].rearrange("p (h f) -> p h f", f=F),
                    mybir.AxisListType.X, ALU.add)
                sumq2 = rot2.tile([P, HPC], F32, tag="sumq2")
                nc.vector.tensor_reduce(
                    sumq2[:], sq2[:].rearrange("p (h f) -> p h f", f=F),
                    mybir.AxisListType.X, ALU.add)
                qsum = rot2.tile([P, HPC], F32, tag="qsum")
                nc.vector.tensor_scalar(qsum[:], sumq2[:], 0.5, 1.0,
                                        ALU.mult, ALU.add)
                nc.vector.tensor_add(qsum[:], qsum[:], sumq[:])
                rq = rot2.tile([P, HPC], F32, tag="rq")
                nc.vector.reciprocal(rq[:], qsum[:])
                nc.vector.tensor_mul(
                    rq[:], rq[:], mask_sb[:, t:t + 1].broadcast_to([P, HPC]))
                rq05 = rot2.tile([P, HPC], F32, tag="rq05")
                nc.vector.tensor_scalar_mul(rq05[:], rq[:], 0.5)

                # phi_q stash: slot0 + square slots on gpsimd (off critical
                # path), linear slots on DVE.
                pq = phiq[:, t]
                nc.gpsimd.tensor_copy(pq[:, :, 0:1], rq[:].unsqueeze(2))
                nc.vector.tensor_mul(
                    pq[:, :, 1:1 + F],
                    qf[:].rearrange("p (h f) -> p h f", f=F),
                    rq[:].unsqueeze(2).broadcast_to([P, HPC, F]))
                nc.gpsimd.tensor_mul(
                    pq[:, :, 1 + F:E],
                    sq2[:].rearrange("p (h f) -> p h f", f=F),
                    rq05[:].unsqueeze(2).broadcast_to([P, HPC, F]))

                # ---- k side (Act) ----
                phik = rot2.tile([P, HPC, E], BF16, tag="phik")
                nc.vector.memset(phik[:, :, 0:1], 1.0)
                nc.scalar.activation(
                    phik[:, :, 1:1 + F],
                    kv1_ps[:, 0:KW].rearrange("p (h f) -> p h f", f=F), AF.Copy)
                nc.scalar.activation(
                    phik[:, :, 1 + F:E],
                    kv1_ps[:, 0:KW].rearrange("p (h f) -> p h f", f=F),
                    AF.Square, scale=RSQRT2)
                # k^2 accumulation for ksum (fp32, off critical path)
                sqk = rot2.tile([P, KW], F32, tag="sqk")
                nc.scalar.activation(sqk[:], kv1_ps[:, 0:KW], AF.Square)
                nc.gpsimd.tensor_add(sqk_acc[:], sqk_acc[:], sqk[:])
                # v16
                v16 = rot2.tile([P, VW], BF16, tag="v16")
                nc.scalar.activation(v16[:, 0:512 - KW], kv1_ps[:, KW:512],
                                     AF.Copy)
                nc.scalar.activation(v16[:, 512 - KW:VW], kv2_ps[:], AF.Copy)
                phik_t[t], v16_t[t] = phik, v16

            def stage_KV(t):
                """kv accumulation matmuls for tile t."""
                phik, v16 = phik_t.pop(t), v16_t.pop(t)
                for h in range(HPC):
                    nc.tensor.matmul(
                        kv_acc[h // 4][0:E, (h % 4) * E:(h % 4) * E + E],
                        phik[:, h, :], v16[:, h * E:h * E + E],
                        start=(t == 0 and h % 4 == 0), stop=(t == NT - 1),
                        skip_group_check=True)

            # late constants (consumed in mid / pass B) are DMA'd from the
            # middle of pass A so they don't delay the first projections.
            late_consts = {}

            def emit_late_consts():
                wkh_sb = consts.tile([P, NCH, 2 * KW], F32R)
                nc.sync.dma_start(out=wkh_sb,
                                  in_=wk_hilo.rearrange("c p j -> p c j"))
                wo_sb = consts.tile([P, OCH, D], BF16)
                nc.sync.dma_start(out=wo_sb,
                                  in_=wo.rearrange("c p j -> p c j"))
                id32_sb = consts.tile([P, P], F32)
                nc.sync.dma_start(out=id32_sb, in_=id32)
                id16_sb = consts.tile([P, P], BF16)
                nc.sync.dma_start(out=id16_sb, in_=id16)
                ones_sb = consts.tile([P, 1], F32)
                nc.sync.dma_start(out=ones_sb, in_=ones_col)
                late_consts.update(wkh_sb=wkh_sb, wo_sb=wo_sb, id32_sb=id32_sb,
                                   id16_sb=id16_sb, ones_sb=ones_sb)

            # =============== PASS A (software pipelined) ===============
            for t in range(NT + 2):
                with nc.named_scope(f"A{t}"):
                    if t == NT // 2:
                        emit_late_consts()
                    if t < NT:
                        stage_T(t)
                    if t == 0:
                        emit_early_consts()
                    if 1 <= t <= NT:
                        stage_P(t - 1)
                    if t >= 2:
                        stage_KV(t - 2)

            wkh_sb = late_consts["wkh_sb"]
            wo_sb = late_consts["wo_sb"]
            id32_sb = late_consts["id32_sb"]
            id16_sb = late_consts["id16_sb"]
            ones_sb = late_consts["ones_sb"]

            # =============== PASS B (software pipelined) ===============
            phiT_t = {}
            osb_t = {}
            oT_t = {}

            def stage_BT(t):
                phiT = rot4.tile([E, HPC, P], BF16, tag="phiT")
                for g in range(2):
                    tp = ps2.tile([P, 512], BF16, tag="kv1", name=f"ptp_{t}_{g}")
                    for hh in range(4):
                        h = g * 4 + hh
                        nc.tensor.transpose(tp[0:E, hh * P:hh * P + P],
                                            phiq[:, t, h, :], id16_sb[:])
                    nc.vector.tensor_copy(
                        phiT[:, g * 4:(g + 1) * 4, :].rearrange(
                            "p h n -> p (h n)"),
                        tp[0:E, :])
                phiT_t[t] = phiT

            def stage_BQ(t):
                phiT = phiT_t.pop(t)
                o_ps = [ps.tile([P, 4 * E], F32, tag="a0", name=f"o0_{t}"),
                        ps.tile([P, 4 * E], F32, tag="a1", name=f"o1_{t}")]
                for h in range(HPC):
                    nc.tensor.matmul(
                        o_ps[h // 4][:, (h % 4) * E:(h % 4) * E + E],
                        phiT[:, h, :], kvs_sb[:, h * E:h * E + E],
                        start=(h % 4 == 0), stop=(h % 4 == 3),
                        skip_group_check=True)
                o_sb = rot2.tile([P, VW], BF16, tag="osb")
                nc.vector.tensor_copy(o_sb[:, 0:4 * E], o_ps[0][:])
                nc.scalar.activation(o_sb[:, 4 * E:VW], o_ps[1][:], AF.Copy)
                osb_t[t] = o_sb

            def stage_BO(t):
                o_sb = osb_t.pop(t)
                oT = rot2.tile([P, OCH, P], BF16, tag="oT")
                for g, cs in enumerate([range(0, 4), range(4, 5)]):
                    w = len(cs) * P
                    tp = ps2.tile([P, 512], BF16, tag="kv2", name=f"otp_{t}_{g}")
                    for c in cs:
                        kk = OLAST if c == OCH - 1 else P
                        nc.tensor.transpose(
                            tp[0:kk, (c % 4) * P:(c % 4) * P + P],
                            o_sb[:, c * P:c * P + kk], id16_sb[:])
                    lo, hi = cs[0], cs[-1] + 1
                    kk = OLAST if hi == OCH else P
                    nc.vector.tensor_copy(
                        oT[0:kk, lo:hi, :].rearrange("p c n -> p (c n)"),
                        tp[0:kk, 0:w])
                oT_t[t] = oT

            def stage_BW(t):
                oT = oT_t.pop(t)
                f1 = ps2.tile([P, 512], F32, tag="qk", name=f"f1_{t}")
                f2 = ps2.tile([P, 512], F32, tag="qk", name=f"f2_{t}")
                f3 = ps2.tile([P, D - 1024], F32, tag="kv2", name=f"f3_{t}")
                for c in range(OCH):
                    kk = OLAST if c == OCH - 1 else P
                    nc.tensor.matmul(f1[:], oT[0:kk, c, :],
                                     wo_sb[0:kk, c, 0:512],
                                     start=(c == 0), stop=(c == OCH - 1))
                    nc.tensor.matmul(f2[:], oT[0:kk, c, :],
                                     wo_sb[0:kk, c, 512:1024],
                                     start=(c == 0), stop=(c == OCH - 1))
                    nc.tensor.matmul(f3[:], oT[0:kk, c, :],
                                     wo_sb[0:kk, c, 1024:D],
                                     start=(c == 0), stop=(c == OCH - 1))
                out_sb = rot2.tile([P, D], F32, tag="outsb")
                nc.vector.tensor_copy(out_sb[:, 0:512], f1[:])
                nc.scalar.activation(out_sb[:, 512:1024], f2[:], AF.Copy)
                nc.vector.tensor_copy(out_sb[:, 1024:D], f3[:])
                nc.sync.dma_start(out=out[t * P:(t + 1) * P, :], in_=out_sb)

            # fill the mid-phase PE idle with the first pass-B transposes
            for t0 in range(3):
                with nc.named_scope(f"B{t0}pre"):
                    stage_BT(t0)

            # =============== MID: ksum assembly ===============
            with nc.named_scope("mid"):
                hs_hi = state.tile([P, NCH], F32R)
                hs_lo = state.tile([P, NCH], F32R)
                nc.vector.tensor_copy(hs_hi[:], hssum_acc[:])
                nc.vector.tensor_tensor(hs_lo[:], hssum_acc[:],
                                        hs_hi[:].bitcast(F32), ALU.subtract)
                # ksum linear slots: 3-term f32r matvec hssum @ Wk
                kl_ps = ps2.tile([1, KW], F32, tag="qk", name="kl")
                for pi, (vec, off) in enumerate(
                        [(hs_hi, 0), (hs_hi, KW), (hs_lo, 0)]):
                    for c in range(NCH):
                        kk = _chunk_k(c)
                        nc.tensor.matmul(
                            kl_ps[:], vec[0:kk, c:c + 1],
                            wkh_sb[0:kk, c, off:off + KW],
                            start=(pi == 0 and c == 0),
                            stop=(pi == 2 and c == NCH - 1),
                            skip_group_check=True)
                # ksum square slots: column-sum sqk_acc over tokens, transpose
                kcol_ps = ps2.tile([P, 2], F32, tag="kv2", name="kcol")
                for j in range(2):
                    nc.tensor.matmul(kcol_ps[:, j:j + 1],
                                     sqk_acc[:, j * P:(j + 1) * P], ones_sb[:],
                                     start=(j == 0), stop=(j == 1),
                                     skip_group_check=True)
                kcol_sb = state.tile([P, 2], F32)
                nc.vector.tensor_copy(kcol_sb[:], kcol_ps[:])
                ktr_ps = ps2.tile([2, P], F32, tag="kv1", name="ktr")
                nc.tensor.transpose(ktr_ps[:], kcol_sb[:], id32_sb[:])
                kt = state.tile([2, P], F32)
                nc.vector.tensor_scalar_mul(kt[:], ktr_ps[:], 0.5)

                krow = ksum_row[:].rearrange("o (h e) -> o h e", e=E)
                nc.vector.memset(krow[:, :, 0:1], float(S))
                nc.vector.tensor_copy(
                    krow[:, :, 1:1 + F],
                    kl_ps[:].rearrange("o (h f) -> o h f", f=F))
                nc.vector.tensor_copy(
                    krow[:, 0:4, 1 + F:E],
                    kt[0:1, :].rearrange("o (h f) -> o h f", f=F))
                nc.sync.dma_start(
                    out=krow[:, 4:8, 1 + F:E],
                    in_=kt[1:2, :].rearrange("o (h f) -> o h f", f=F))
                nc.vector.reciprocal(rk_row[:], ksum_row[:])
                nc.gpsimd.partition_broadcast(rk_bc[:], rk_row[:])
                for i in range(2):
                    nc.vector.tensor_mul(
                        kvs_sb[:, i * 4 * E:(i + 1) * 4 * E],
                        kv_acc[i][0:E, :],
                        rk_bc[0:E, i * 4 * E:(i + 1) * 4 * E])

            for j in range(NT + 2):
                with nc.named_scope(f"B{j}"):
                    if j < NT:
                        stage_BQ(j)
                    if j + 3 < NT:
                        stage_BT(j + 3)
                    if 1 <= j <= NT:
                        stage_BO(j - 1)
                    if 2 <= j <= NT + 1:
                        stage_BW(j - 2)


    nc.compile()
    return nc


def _r12(x):
    """Round fp32 mantissa to 12 explicit bits (safe under PE f32r reads)."""
    xi = np.ascontiguousarray(x, dtype=np.float32).view(np.uint32)
    out = ((xi + np.uint32(0x800)) & np.uint32(0xFFFFF000)).view(np.float32)
    return out.copy()


def _chunks(w):
    out = np.zeros((NCH, P, w.shape[1]), dtype=np.float32)
    for c in range(NCH):
        kk = _chunk_k(c)
        out[c, 0:kk] = w[c * P:c * P + kk]
    return out


def _prep_core_inputs(hidden_states, attention_mask, Wq, Wk, Wv, Wo, core):
    b, half = core // 2, core % 2
    h0 = half * HPC
    bf = ml_dtypes.bfloat16

    # transposed, zero-padded to 1152 rows, chunked [NCH, P, S]
    hsT = np.zeros((NCH, P, S), dtype=np.float32)
    hsTf = np.ascontiguousarray(hidden_states[b].astype(np.float32).T)
    for c in range(NCH):
        kk = _chunk_k(c)
        hsT[c, 0:kk] = hsTf[c * P:c * P + kk]
    maskf = np.ascontiguousarray(
        attention_mask[b].astype(np.float32).reshape(NT, P).T)

    wq_h = Wq[:, h0 * F:(h0 + HPC) * F].astype(np.float32)
    wk_h = Wk[:, h0 * F:(h0 + HPC) * F].astype(np.float32)
    wv_h = Wv[:, h0 * E:(h0 + HPC) * E].astype(np.float32)
    wq_hi = _r12(wq_h)
    wk_hi = _r12(wk_h)
    wq_hilo = _chunks(np.concatenate([wq_hi, wq_h - wq_hi], axis=1))
    wk_hilo = _chunks(np.concatenate([wk_hi, wk_h - wk_hi], axis=1))
    wkv = _chunks(np.concatenate([wk_h, wv_h], axis=1)).astype(bf)

    wo_rows = Wo[h0 * E:(h0 + HPC) * E].astype(np.float32)
    wo_h = np.zeros((OCH, P, D), dtype=np.float32)
    for c in range(OCH):
        kk = OLAST if c == OCH - 1 else P
        wo_h[c, 0:kk] = wo_rows[c * P:c * P + kk]
    wo_h = wo_h.astype(bf)

    return {
        "hsT": hsT,
        "maskf": maskf,
        "wq_hilo": wq_hilo,
        "wk_hilo": wk_hilo,
        "wkv": wkv,
        "wo": wo_h,
        "id32": np.eye(P, dtype=np.float32),
        "id16": np.eye(P, dtype=np.float32).astype(bf),
        "ones_col": np.ones((P, 1), dtype=np.float32),
    }


def kernel(hidden_states, attention_mask, Wq, Wk, Wv, Wo, _trace=False):
    hidden_states = np.asarray(hidden_states)
    attention_mask = np.asarray(attention_mask)
    Wq = np.asarray(Wq); Wk = np.asarray(Wk)
    Wv = np.asarray(Wv); Wo = np.asarray(Wo)

    if "nc" not in _CACHED:
        _CACHED["nc"] = build_bass()
    nc = _CACHED["nc"]

    in_maps = [
        _prep_core_inputs(hidden_states, attention_mask, Wq, Wk, Wv, Wo, c)
        for c in range(8)
    ]
    res = run_bass_kernel_spmd(nc, in_maps, core_ids=list(range(8)),
                               trace=_trace)
    _CACHED["last_result"] = res
    out = np.empty((B, S, D), dtype=np.float32)
    for b in range(B):
        out[b] = res.results[2 * b]["out"] + res.results[2 * b + 1]["out"]
    return out
